# revision 12
# baseline (speedup 1.0000x reference)
"""Bass/Tile Trainium2 kernel for nn_Bi_lstm_46780783788462.

LSTM (H=32, I=3, S=1024) + relu-softmax attention pooling + 2-layer FC head,
data-parallel over batch B=2048 across 8 NeuronCores (256 batch per core).

Layout: gates on partitions ([4H=128, B] per step), batch on the free dim.
All gate nonlinearities are computed with a single Tanh activation per
batch-group using sigmoid(x) = (1 + tanh(x/2))/2; the resulting factor-2
scale is absorbed by storing the cell state doubled (c~ = 2c) and the hidden
state doubled (h2 = 2h, bf16), with compensating 0.5 factors folded into
W_hh, the attention weights and the pooling reduction matrix on the host.

The attention softmax is deferred: h2 for every step is kept in SBUF (bf16,
16 MiB) and phase 2 computes scores / exp / weighted pooling with batched
matmuls, using exp(relu(s)) == max(exp(s), 1).
"""

import sys

if "/opt/trn_rl_repo" not in sys.path:
    sys.path.insert(0, "/opt/trn_rl_repo")

from contextlib import ExitStack

import numpy as np
import ml_dtypes

import concourse.bass as bass
import concourse.bacc as bacc
import concourse.tile as tile
from concourse import mybir
from concourse.bass_utils import run_bass_kernel_spmd

F32 = mybir.dt.float32
BF16 = mybir.dt.bfloat16
FP16 = mybir.dt.float16
AF = mybir.ActivationFunctionType
OP = mybir.AluOpType

H = 32
I_DIM = 3
OUT = 2
NCORES = 8
BL = 256          # batch per core
GB = 128          # batch-group width (free-dim split for pipelining)
NG = BL // GB     # 2 groups
TW = 32           # x window length (timesteps per DMA)

# gate row permutation: torch order [i, f, g, o] -> ours [i, f, o, g]
PERM = np.concatenate([np.arange(0, 64), np.arange(96, 128), np.arange(64, 96)])


def build_program(S: int = 1024):
    """Build + compile the per-core Bass program (identical on all cores)."""
    nc = bacc.Bacc(
        "TRN2", target_bir_lowering=False, debug=False, num_devices=NCORES
    )

    xT = nc.declare_dram_parameter("xT", [I_DIM, S * BL], FP16, isOutput=False)
    w_ih = nc.declare_dram_parameter("w_ih", [I_DIM, 4 * H], FP16, isOutput=False)
    w_hh = nc.declare_dram_parameter("w_hh", [4 * H, 4 * H], FP16, isOutput=False)
    w_hhz = nc.declare_dram_parameter("w_hhz", [4 * H, 4 * H], FP16, isOutput=False)
    scale_v = nc.declare_dram_parameter("scale_v", [4 * H, 1], F32, isOutput=False)
    bias_v = nc.declare_dram_parameter("bias_v", [4 * H, 1], F32, isOutput=False)
    attn_bc = nc.declare_dram_parameter("attn_bc", [128, 128], FP16, isOutput=False)
    sum4 = nc.declare_dram_parameter("sum4", [128, H], FP16, isOutput=False)
    dsel = nc.declare_dram_parameter("dsel", [128, 1], FP16, isOutput=False)
    fc1w = nc.declare_dram_parameter("fc1w", [H, 16], F32, isOutput=False)
    fc1b = nc.declare_dram_parameter("fc1b", [16, 1], F32, isOutput=False)
    fc2w = nc.declare_dram_parameter("fc2w", [16, OUT], F32, isOutput=False)
    fc2b = nc.declare_dram_parameter("fc2b", [OUT, 1], F32, isOutput=False)
    ones_bc = nc.declare_dram_parameter("ones_bc", [1, H], F32, isOutput=False)
    out = nc.declare_dram_parameter("out", [BL, OUT], F32, isOutput=True)

    with tile.TileContext(nc) as tc:
        with ExitStack() as ctx:
            _body(ctx, tc, S, xT, w_ih, w_hh, w_hhz, scale_v, bias_v, attn_bc,
                  sum4, dsel, fc1w, fc1b, fc2w, fc2b, ones_bc, out)

    nc.compile()
    return nc


def _body(ctx, tc, S, xT, w_ih, w_hh, w_hhz, scale_v, bias_v, attn_bc, sum4,
          dsel, fc1w, fc1b, fc2w, fc2b, ones_bc, out):
    nc = tc.nc
    singles = ctx.enter_context(tc.tile_pool(name="singles", bufs=1))

    # persistent SBUF tensors
    hs_buf = singles.tile([128, (S // 4) * BL], FP16)  # h2 history, packed 4 steps/partition-block
    c_st = singles.tile([2 * H, BL], FP16)             # c~ = 2c lives on rows 32:64
    w_ih_sb = singles.tile([I_DIM, 4 * H], FP16)
    w_hh_sb = singles.tile([4 * H, 4 * H], FP16)
    w_hhz_sb = singles.tile([4 * H, 4 * H], FP16)
    scale_sb = singles.tile([4 * H, 1], F32)
    bias_sb = singles.tile([4 * H, 1], F32)
    attn_sb = singles.tile([128, 128], FP16)
    sum4_sb = singles.tile([128, H], FP16)
    dsel_sb = singles.tile([128, 1], FP16)
    fc1w_sb = singles.tile([H, 16], F32)
    fc1b_sb = singles.tile([16, 1], F32)
    fc2w_sb = singles.tile([16, OUT], F32)
    fc2b_sb = singles.tile([OUT, 1], F32)
    ones_sb = singles.tile([1, H], F32)

    for dst, src in [(w_ih_sb, w_ih), (w_hh_sb, w_hh), (w_hhz_sb, w_hhz),
                     (scale_sb, scale_v),
                     (bias_sb, bias_v), (attn_sb, attn_bc), (sum4_sb, sum4),
                     (dsel_sb, dsel), (fc1w_sb, fc1w), (fc1b_sb, fc1b),
                     (fc2w_sb, fc2w), (fc2b_sb, fc2b), (ones_sb, ones_bc)]:
        nc.sync.dma_start(out=dst[:], in_=src[:])

    nc.vector.memset(c_st[32:64, :], 0.0)

    # ---------------- phase 1: LSTM recurrence ----------------
    with (
        tc.tile_pool(name="xw", bufs=2) as xwp,
        tc.tile_pool(name="gpsum", bufs=6, space=bass.MemorySpace.PSUM) as gp,
        tc.tile_pool(name="gates", bufs=6) as gtp,
    ):
        xw = None
        for t in range(S):
            if t % TW == 0:
                xw = xwp.tile([I_DIM, TW * BL], FP16)
                nc.sync.dma_start(
                    out=xw[:], in_=xT[:, t * BL:(t + TW) * BL]
                )
            sl = t % TW
            if t % 2 == 0:
                G2 = gp.tile([128, 2 * BL], F32)
                nc.tensor.matmul(G2[:], w_ih_sb[:],
                                 xw[:, sl * BL:(sl + 2) * BL],
                                 start=True, stop=False)
            G = G2[:, (t % 2) * BL:(t % 2 + 1) * BL]
            t_all = gtp.tile([3 * H, BL], FP16)
            g_t = gtp.tile([H, BL], FP16)
            u2 = gtp.tile([2 * H, BL], FP16)
            a4 = gtp.tile([2 * H, BL], FP16)
            tc_t = gtp.tile([3 * H, BL], FP16)
            hr = 32 * (t % 4)
            hc = (t // 4) * BL
            pr = 32 * ((t - 1) % 4)
            pc = ((t - 1) // 4) * BL
            for g in range(NG):
                gc = slice(g * GB, (g + 1) * GB)
                if t > 0:
                    if pr == 96:
                        # PE can't address base partition 96: read K=64 from
                        # offset 64 with zero-padded weights on rows 64:96.
                        nc.tensor.matmul(
                            G[:, gc], w_hhz_sb[64:128, :],
                            hs_buf[64:128, pc + g * GB: pc + (g + 1) * GB],
                            start=False, stop=True)
                    else:
                        nc.tensor.matmul(
                            G[:, gc], w_hh_sb[pr:pr + 32, :],
                            hs_buf[pr:pr + 32, pc + g * GB: pc + (g + 1) * GB],
                            start=False, stop=True)
                # gtilde = tanh(G_g + b_g), remapped to base partition 0
                nc.scalar.activation(g_t[:, gc], G[96:128, gc], AF.Tanh,
                                     bias=bias_sb[96:128, :])
                # t_all rows [i@0, f@32, o@64] = tanh(0.5*G + 0.5*b) = 2*sig - 1
                nc.scalar.activation(t_all[:, gc], G[0:96, gc], AF.Tanh,
                                     bias=bias_sb[0:96, :],
                                     scale=scale_sb[0:96, :])
                # u2 = (t_i + 1) * gtilde = 2 * i * gtilde
                nc.vector.scalar_tensor_tensor(
                    u2[32:64, gc], t_all[0:32, gc], 1.0, g_t[:, gc],
                    op0=OP.add, op1=OP.mult)
                # a4 = (t_f + 1) * c~ = 4 * f * c
                nc.vector.scalar_tensor_tensor(
                    a4[32:64, gc], t_all[32:64, gc], 1.0, c_st[32:64, gc],
                    op0=OP.add, op1=OP.mult)
                # c~ = 0.5*a4 + u2 = 2 * (f*c + i*gtilde)
                nc.vector.scalar_tensor_tensor(
                    c_st[32:64, gc], a4[32:64, gc], 0.5, u2[32:64, gc],
                    op0=OP.mult, op1=OP.add)
                # tanh(c) = tanh(0.5 * c~), remapped to rows 64:96 to pair with o
                nc.scalar.activation(tc_t[64:96, gc], c_st[32:64, gc], AF.Tanh,
                                     scale=0.5)
                # h2 = (t_o + 1) * tanh(c) = 2 * o * tanh(c)
                nc.vector.scalar_tensor_tensor(
                    hs_buf[hr:hr + 32, hc + g * GB: hc + (g + 1) * GB],
                    t_all[64:96, gc], 1.0, tc_t[64:96, gc],
                    op0=OP.add, op1=OP.mult)

    # ---------------- phase 2: scores + exp + weighted pooling ----------------
    with tc.tile_pool(name="acc", bufs=1, space=bass.MemorySpace.PSUM) as accp:
        pooled_ps = accp.tile([H, BL], F32)
        d_ps = accp.tile([1, BL], F32)
        NCH = (S // 4) * BL // 512
        with (
            tc.tile_pool(name="p2psum", bufs=2, space=bass.MemorySpace.PSUM) as pp2,
            tc.tile_pool(name="p2sb", bufs=3) as p2,
        ):
            for ch in range(NCH):
                cc = slice(ch * 512, (ch + 1) * 512)
                s_bc = pp2.tile([128, 512], F32)
                nc.tensor.matmul(s_bc[:], attn_sb[:], hs_buf[:, cc],
                                 start=True, stop=True)
                e_exp = p2.tile([128, 512], FP16)
                nc.scalar.activation(e_exp[:], s_bc[:], AF.Exp)
                emax = p2.tile([128, 512], FP16)
                nc.vector.tensor_scalar_max(emax[:], e_exp[:], 1.0)
                nc.vector.tensor_mul(hs_buf[:, cc], hs_buf[:, cc], emax[:])
                for hf in range(2):
                    c0 = ch * 512 + hf * 256
                    nc.tensor.matmul(pooled_ps[:], sum4_sb[:],
                                     hs_buf[:, c0:c0 + 256],
                                     start=(ch == 0 and hf == 0),
                                     stop=(ch == NCH - 1 and hf == 1))
                for hf in range(2):
                    nc.tensor.matmul(d_ps[:], dsel_sb[:],
                                     emax[:, hf * 256:(hf + 1) * 256],
                                     start=(ch == 0 and hf == 0),
                                     stop=(ch == NCH - 1 and hf == 1))

        # ---------------- phase 3: normalize + FC head ----------------
        with (
            tc.tile_pool(name="p3psum", bufs=1, space=bass.MemorySpace.PSUM) as pp3,
            tc.tile_pool(name="p3sb", bufs=1) as p3,
        ):
            d_sb = p3.tile([1, BL], F32)
            nc.vector.tensor_copy(d_sb[:], d_ps[:])
            rd = p3.tile([1, BL], F32)
            nc.vector.reciprocal(rd[:], d_sb[:])
            rdb_ps = pp3.tile([H, BL], F32)
            nc.tensor.matmul(rdb_ps[:], ones_sb[:], rd[:], start=True, stop=True)
            pooled_sb = p3.tile([H, BL], F32)
            nc.vector.tensor_copy(pooled_sb[:], pooled_ps[:])
            pooln = p3.tile([H, BL], F32)
            nc.vector.tensor_mul(pooln[:], pooled_sb[:], rdb_ps[:])
            h1_ps = pp3.tile([16, BL], F32)
            nc.tensor.matmul(h1_ps[:], fc1w_sb[:], pooln[:], start=True, stop=True)
            h1 = p3.tile([16, BL], F32)
            nc.scalar.activation(h1[:], h1_ps[:], AF.Relu, bias=fc1b_sb[:])
            o_ps = pp3.tile([OUT, BL], F32)
            nc.tensor.matmul(o_ps[:], fc2w_sb[:], h1[:], start=True, stop=True)
            o_sb = p3.tile([OUT, BL], F32)
            nc.vector.tensor_scalar_add(o_sb[:], o_ps[:], fc2b_sb[:])
            nc.sync.dma_start(out=out[:].rearrange("b o -> o b"), in_=o_sb[:])


def make_host_inputs(x, W_ih, W_hh, b_ih, b_hh, attn_w, fc1_w, fc1_b,
                     fc2_w, fc2_b, S):
    """Host-side weight preprocessing shared by all cores (core-independent)."""
    bf16 = ml_dtypes.bfloat16
    fp16 = np.float16
    Wih_p = W_ih[PERM]                       # [128, 3]
    Whh_p = W_hh[PERM]                       # [128, 32]
    b_p = (b_ih + b_hh)[PERM]                # [128]
    scale_vec = np.where(np.arange(128) < 96, 0.5, 1.0).astype(np.float32)
    bias_vec = (b_p * scale_vec).astype(np.float32)

    attn_blk = np.zeros((128, 128), np.float32)
    for tm in range(4):
        attn_blk[32 * tm:32 * tm + 32, 32 * tm:32 * tm + 32] = np.tile(
            0.5 * attn_w.reshape(H, 1), (1, 32))
    sum4_m = np.tile(0.5 * np.eye(H, dtype=np.float32), (4, 1))   # [128, 32]
    dsel_m = np.zeros((128, 1), np.float32)
    dsel_m[::32, 0] = 1.0

    common = {
        "w_ih": np.ascontiguousarray(Wih_p.T).astype(fp16),
        "w_hh": np.tile(np.ascontiguousarray(0.5 * Whh_p.T), (4, 1)).astype(fp16),
        "w_hhz": np.concatenate([
            np.zeros((96, 128), np.float32),
            np.ascontiguousarray(0.5 * Whh_p.T)]).astype(fp16),
        "scale_v": scale_vec.reshape(128, 1),
        "bias_v": bias_vec.reshape(128, 1),
        "attn_bc": attn_blk.astype(fp16),
        "sum4": sum4_m.astype(fp16),
        "dsel": dsel_m.astype(fp16),
        "fc1w": np.ascontiguousarray(fc1_w.T).astype(np.float32),
        "fc1b": fc1_b.reshape(16, 1).astype(np.float32),
        "fc2w": np.ascontiguousarray(fc2_w.T).astype(np.float32),
        "fc2b": fc2_b.reshape(OUT, 1).astype(np.float32),
        "ones_bc": np.ones((1, H), np.float32),
    }
    in_maps = []
    for c in range(NCORES):
        xc = x[c * BL:(c + 1) * BL]                     # [BL, S, 3]
        xT_c = np.ascontiguousarray(xc.transpose(2, 1, 0)).reshape(I_DIM, S * BL)
        in_maps.append({"xT": xT_c.astype(fp16), **common})
    return in_maps


_CACHE = {}


def _get_program(S):
    if S not in _CACHE:
        _CACHE[S] = build_program(S)
    return _CACHE[S]


def run(inputs, S=1024, trace=False):
    if trace:
        # no S3 in this container; keep NTFF processing local
        import concourse.bass_utils as bu
        bu.upload_artifacts = lambda tmpdir: str(tmpdir)
    nc = _get_program(S)
    in_maps = make_host_inputs(
        inputs["x"], inputs["W_ih"], inputs["W_hh"], inputs["b_ih"],
        inputs["b_hh"], inputs["attn_w"], inputs["fc1_w"], inputs["fc1_b"],
        inputs["fc2_w"], inputs["fc2_b"], S)
    res = run_bass_kernel_spmd(
        nc, in_maps, core_ids=list(range(NCORES)), trace=trace)
    outs = np.concatenate([r["out"] for r in res.results], axis=0)
    return outs.astype(np.float32), res


def kernel(**inputs):
    out, _ = run(inputs, S=int(inputs["x"].shape[1]))
    return out


# revision 17
# speedup vs baseline: 1.1358x; 1.1358x over previous
"""Bass/Tile Trainium2 kernel for nn_Bi_lstm_46780783788462.

LSTM (H=32, I=3, S=1024) + relu-softmax attention pooling + 2-layer FC head,
data-parallel over batch B=2048 across 8 NeuronCores (256 batch per core).

Layout: gates on partitions ([4H=128, B] per step), batch on the free dim.
All gate nonlinearities are computed with a single Tanh activation per
batch-group using sigmoid(x) = (1 + tanh(x/2))/2; the resulting factor-2
scale is absorbed by storing the cell state doubled (c~ = 2c) and the hidden
state doubled (h2 = 2h, bf16), with compensating 0.5 factors folded into
W_hh, the attention weights and the pooling reduction matrix on the host.

The attention softmax is deferred: h2 for every step is kept in SBUF (bf16,
16 MiB) and phase 2 computes scores / exp / weighted pooling with batched
matmuls, using exp(relu(s)) == max(exp(s), 1).
"""

import sys

if "/opt/trn_rl_repo" not in sys.path:
    sys.path.insert(0, "/opt/trn_rl_repo")

from contextlib import ExitStack

import numpy as np
import ml_dtypes

import concourse.bass as bass
import concourse.bacc as bacc
import concourse.tile as tile
from concourse import mybir
from concourse.bass_utils import run_bass_kernel_spmd

F32 = mybir.dt.float32
BF16 = mybir.dt.bfloat16
FP16 = mybir.dt.float16
AF = mybir.ActivationFunctionType
OP = mybir.AluOpType

H = 32
I_DIM = 3
OUT = 2
NCORES = 8
BL = 256          # batch per core
GB = 128          # batch-group width (free-dim split for pipelining)
NG = BL // GB     # 2 groups
TW = 32           # x window length (timesteps per DMA)

# gate row permutation: torch order [i, f, g, o] -> ours [i, f, o, g]
PERM = np.concatenate([np.arange(0, 64), np.arange(96, 128), np.arange(64, 96)])


def build_program(S: int = 1024):
    """Build + compile the per-core Bass program (identical on all cores)."""
    nc = bacc.Bacc(
        "TRN2", target_bir_lowering=False, debug=False, num_devices=NCORES
    )

    xT = nc.declare_dram_parameter("xT", [I_DIM, S * BL], FP16, isOutput=False)
    w_ih = nc.declare_dram_parameter("w_ih", [I_DIM, 4 * H], FP16, isOutput=False)
    w_hh = nc.declare_dram_parameter("w_hh", [4 * H, 4 * H], FP16, isOutput=False)
    w_hhz = nc.declare_dram_parameter("w_hhz", [4 * H, 4 * H], FP16, isOutput=False)
    scale_v = nc.declare_dram_parameter("scale_v", [4 * H, 1], F32, isOutput=False)
    bias_v = nc.declare_dram_parameter("bias_v", [4 * H, 1], F32, isOutput=False)
    attn_bc = nc.declare_dram_parameter("attn_bc", [128, 128], FP16, isOutput=False)
    sum4 = nc.declare_dram_parameter("sum4", [128, H], FP16, isOutput=False)
    dsel = nc.declare_dram_parameter("dsel", [128, 1], FP16, isOutput=False)
    fc1w = nc.declare_dram_parameter("fc1w", [H, 16], F32, isOutput=False)
    fc1b = nc.declare_dram_parameter("fc1b", [16, 1], F32, isOutput=False)
    fc2w = nc.declare_dram_parameter("fc2w", [16, OUT], F32, isOutput=False)
    fc2b = nc.declare_dram_parameter("fc2b", [OUT, 1], F32, isOutput=False)
    ones_bc = nc.declare_dram_parameter("ones_bc", [1, H], F32, isOutput=False)
    out = nc.declare_dram_parameter("out", [BL, OUT], F32, isOutput=True)

    with tile.TileContext(nc) as tc:
        with ExitStack() as ctx:
            _body(ctx, tc, S, xT, w_ih, w_hh, w_hhz, scale_v, bias_v, attn_bc,
                  sum4, dsel, fc1w, fc1b, fc2w, fc2b, ones_bc, out)

    nc.compile()
    return nc


def _body(ctx, tc, S, xT, w_ih, w_hh, w_hhz, scale_v, bias_v, attn_bc, sum4,
          dsel, fc1w, fc1b, fc2w, fc2b, ones_bc, out):
    nc = tc.nc
    singles = ctx.enter_context(tc.tile_pool(name="singles", bufs=1))

    # persistent SBUF tensors
    hs_buf = singles.tile([128, (S // 4) * BL], FP16)  # h2 history, packed 4 steps/partition-block
    c_st = singles.tile([2 * H, BL], FP16)             # c~ = 2c lives on rows 32:64
    w_ih_sb = singles.tile([I_DIM, 4 * H], FP16)
    w_hh_sb = singles.tile([4 * H, 4 * H], FP16)
    w_hhz_sb = singles.tile([4 * H, 4 * H], FP16)
    scale_sb = singles.tile([4 * H, 1], F32)
    bias_sb = singles.tile([4 * H, 1], F32)
    attn_sb = singles.tile([128, 128], FP16)
    sum4_sb = singles.tile([128, H], FP16)
    dsel_sb = singles.tile([128, 1], FP16)
    fc1w_sb = singles.tile([H, 16], F32)
    fc1b_sb = singles.tile([16, 1], F32)
    fc2w_sb = singles.tile([16, OUT], F32)
    fc2b_sb = singles.tile([OUT, 1], F32)
    ones_sb = singles.tile([1, H], F32)

    for dst, src in [(w_ih_sb, w_ih), (w_hh_sb, w_hh), (w_hhz_sb, w_hhz),
                     (scale_sb, scale_v),
                     (bias_sb, bias_v), (attn_sb, attn_bc), (sum4_sb, sum4),
                     (dsel_sb, dsel), (fc1w_sb, fc1w), (fc1b_sb, fc1b),
                     (fc2w_sb, fc2w), (fc2b_sb, fc2b), (ones_sb, ones_bc)]:
        nc.sync.dma_start(out=dst[:], in_=src[:])

    nc.vector.memset(c_st[32:64, :], 0.0)

    # ---------------- phase 1: LSTM recurrence ----------------
    with (
        tc.tile_pool(name="xw", bufs=2) as xwp,
        tc.tile_pool(name="gpsum", bufs=6, space=bass.MemorySpace.PSUM) as gp,
        tc.tile_pool(name="gates", bufs=6) as gtp,
    ):
        xw = None
        for t in range(S):
            if t % TW == 0:
                xw = xwp.tile([I_DIM, TW * BL], FP16)
                nc.sync.dma_start(
                    out=xw[:], in_=xT[:, t * BL:(t + TW) * BL]
                )
            sl = t % TW
            if t % 2 == 0:
                G2 = gp.tile([128, 2 * BL], F32)
                nc.tensor.matmul(G2[:], w_ih_sb[:],
                                 xw[:, sl * BL:(sl + 2) * BL],
                                 start=True, stop=False)
            G = G2[:, (t % 2) * BL:(t % 2 + 1) * BL]
            t_all = gtp.tile([3 * H, BL], FP16)
            g_t = gtp.tile([H, BL], FP16)
            u2 = gtp.tile([2 * H, BL], FP16)
            a4 = gtp.tile([2 * H, BL], FP16)
            tc_t = gtp.tile([3 * H, BL], FP16)
            hr = 32 * (t % 4)
            hc = (t // 4) * BL
            pr = 32 * ((t - 1) % 4)
            pc = ((t - 1) // 4) * BL
            for g in range(NG):
                gc = slice(g * GB, (g + 1) * GB)
                if t > 0:
                    if pr == 96:
                        # PE can't address base partition 96: read K=64 from
                        # offset 64 with zero-padded weights on rows 64:96.
                        nc.tensor.matmul(
                            G[:, gc], w_hhz_sb[64:128, :],
                            hs_buf[64:128, pc + g * GB: pc + (g + 1) * GB],
                            start=False, stop=True)
                    else:
                        nc.tensor.matmul(
                            G[:, gc], w_hh_sb[pr:pr + 32, :],
                            hs_buf[pr:pr + 32, pc + g * GB: pc + (g + 1) * GB],
                            start=False, stop=True)
                # gtilde = tanh(G_g + b_g), remapped to base partition 0
                nc.scalar.activation(g_t[:, gc], G[96:128, gc], AF.Tanh,
                                     bias=bias_sb[96:128, :])
                # s rows [i@0, f@32, o@64] = sigmoid(G + b)
                nc.scalar.activation(t_all[:, gc], G[0:96, gc], AF.Sigmoid,
                                     bias=bias_sb[0:96, :])
                # u = i * gtilde
                nc.vector.tensor_mul(u2[32:64, gc], t_all[0:32, gc],
                                     g_t[:, gc])
                # p = f * c
                nc.vector.tensor_mul(a4[32:64, gc], t_all[32:64, gc],
                                     c_st[32:64, gc])
                # c = p + u
                nc.vector.tensor_add(c_st[32:64, gc], a4[32:64, gc],
                                     u2[32:64, gc])
                # tanh(c), remapped to rows 64:96 to pair with o
                nc.scalar.activation(tc_t[64:96, gc], c_st[32:64, gc], AF.Tanh)
                # h = o * tanh(c)
                nc.vector.tensor_mul(
                    hs_buf[hr:hr + 32, hc + g * GB: hc + (g + 1) * GB],
                    t_all[64:96, gc], tc_t[64:96, gc])

    # ---------------- phase 2: scores + exp + weighted pooling ----------------
    with tc.tile_pool(name="acc", bufs=1, space=bass.MemorySpace.PSUM) as accp:
        pooled_ps = accp.tile([H, BL], F32)
        d_ps = accp.tile([1, BL], F32)
        NCH = (S // 4) * BL // 512
        with (
            tc.tile_pool(name="p2psum", bufs=2, space=bass.MemorySpace.PSUM) as pp2,
            tc.tile_pool(name="p2sb", bufs=3) as p2,
        ):
            for ch in range(NCH):
                cc = slice(ch * 512, (ch + 1) * 512)
                s_bc = pp2.tile([128, 512], F32)
                nc.tensor.matmul(s_bc[:], attn_sb[:], hs_buf[:, cc],
                                 start=True, stop=True)
                e_exp = p2.tile([128, 512], FP16)
                nc.scalar.activation(e_exp[:], s_bc[:], AF.Exp)
                emax = p2.tile([128, 512], FP16)
                nc.vector.tensor_scalar_max(emax[:], e_exp[:], 1.0)
                nc.vector.tensor_mul(hs_buf[:, cc], hs_buf[:, cc], emax[:])
                for hf in range(2):
                    c0 = ch * 512 + hf * 256
                    nc.tensor.matmul(pooled_ps[:], sum4_sb[:],
                                     hs_buf[:, c0:c0 + 256],
                                     start=(ch == 0 and hf == 0),
                                     stop=(ch == NCH - 1 and hf == 1))
                for hf in range(2):
                    nc.tensor.matmul(d_ps[:], dsel_sb[:],
                                     emax[:, hf * 256:(hf + 1) * 256],
                                     start=(ch == 0 and hf == 0),
                                     stop=(ch == NCH - 1 and hf == 1))

        # ---------------- phase 3: normalize + FC head ----------------
        with (
            tc.tile_pool(name="p3psum", bufs=1, space=bass.MemorySpace.PSUM) as pp3,
            tc.tile_pool(name="p3sb", bufs=1) as p3,
        ):
            d_sb = p3.tile([1, BL], F32)
            nc.vector.tensor_copy(d_sb[:], d_ps[:])
            rd = p3.tile([1, BL], F32)
            nc.vector.reciprocal(rd[:], d_sb[:])
            rdb_ps = pp3.tile([H, BL], F32)
            nc.tensor.matmul(rdb_ps[:], ones_sb[:], rd[:], start=True, stop=True)
            pooled_sb = p3.tile([H, BL], F32)
            nc.vector.tensor_copy(pooled_sb[:], pooled_ps[:])
            pooln = p3.tile([H, BL], F32)
            nc.vector.tensor_mul(pooln[:], pooled_sb[:], rdb_ps[:])
            h1_ps = pp3.tile([16, BL], F32)
            nc.tensor.matmul(h1_ps[:], fc1w_sb[:], pooln[:], start=True, stop=True)
            h1 = p3.tile([16, BL], F32)
            nc.scalar.activation(h1[:], h1_ps[:], AF.Relu, bias=fc1b_sb[:])
            o_ps = pp3.tile([OUT, BL], F32)
            nc.tensor.matmul(o_ps[:], fc2w_sb[:], h1[:], start=True, stop=True)
            o_sb = p3.tile([OUT, BL], F32)
            nc.vector.tensor_scalar_add(o_sb[:], o_ps[:], fc2b_sb[:])
            nc.sync.dma_start(out=out[:].rearrange("b o -> o b"), in_=o_sb[:])


def make_host_inputs(x, W_ih, W_hh, b_ih, b_hh, attn_w, fc1_w, fc1_b,
                     fc2_w, fc2_b, S):
    """Host-side weight preprocessing shared by all cores (core-independent)."""
    bf16 = ml_dtypes.bfloat16
    fp16 = np.float16
    Wih_p = W_ih[PERM]                       # [128, 3]
    Whh_p = W_hh[PERM]                       # [128, 32]
    b_p = (b_ih + b_hh)[PERM]                # [128]
    scale_vec = np.ones(128, np.float32)
    bias_vec = b_p.astype(np.float32)

    attn_blk = np.zeros((128, 128), np.float32)
    for tm in range(4):
        attn_blk[32 * tm:32 * tm + 32, 32 * tm:32 * tm + 32] = np.tile(
            attn_w.reshape(H, 1), (1, 32))
    sum4_m = np.tile(np.eye(H, dtype=np.float32), (4, 1))   # [128, 32]
    dsel_m = np.zeros((128, 1), np.float32)
    dsel_m[::32, 0] = 1.0

    common = {
        "w_ih": np.ascontiguousarray(Wih_p.T).astype(fp16),
        "w_hh": np.tile(np.ascontiguousarray(Whh_p.T), (4, 1)).astype(fp16),
        "w_hhz": np.concatenate([
            np.zeros((96, 128), np.float32),
            np.ascontiguousarray(Whh_p.T)]).astype(fp16),
        "scale_v": scale_vec.reshape(128, 1),
        "bias_v": bias_vec.reshape(128, 1),
        "attn_bc": attn_blk.astype(fp16),
        "sum4": sum4_m.astype(fp16),
        "dsel": dsel_m.astype(fp16),
        "fc1w": np.ascontiguousarray(fc1_w.T).astype(np.float32),
        "fc1b": fc1_b.reshape(16, 1).astype(np.float32),
        "fc2w": np.ascontiguousarray(fc2_w.T).astype(np.float32),
        "fc2b": fc2_b.reshape(OUT, 1).astype(np.float32),
        "ones_bc": np.ones((1, H), np.float32),
    }
    in_maps = []
    for c in range(NCORES):
        xc = x[c * BL:(c + 1) * BL]                     # [BL, S, 3]
        xT_c = np.ascontiguousarray(xc.transpose(2, 1, 0)).reshape(I_DIM, S * BL)
        in_maps.append({"xT": xT_c.astype(fp16), **common})
    return in_maps


_CACHE = {}


def _get_program(S):
    if S not in _CACHE:
        _CACHE[S] = build_program(S)
    return _CACHE[S]


def run(inputs, S=1024, trace=False):
    if trace:
        # no S3 in this container; keep NTFF processing local
        import concourse.bass_utils as bu
        bu.upload_artifacts = lambda tmpdir: str(tmpdir)
    nc = _get_program(S)
    in_maps = make_host_inputs(
        inputs["x"], inputs["W_ih"], inputs["W_hh"], inputs["b_ih"],
        inputs["b_hh"], inputs["attn_w"], inputs["fc1_w"], inputs["fc1_b"],
        inputs["fc2_w"], inputs["fc2_b"], S)
    res = run_bass_kernel_spmd(
        nc, in_maps, core_ids=list(range(NCORES)), trace=trace)
    outs = np.concatenate([r["out"] for r in res.results], axis=0)
    return outs.astype(np.float32), res


def kernel(**inputs):
    out, _ = run(inputs, S=int(inputs["x"].shape[1]))
    return out
